# revision 29
# baseline (speedup 1.0000x reference)
"""Trainium2 Bass kernel for an attention-decoder LSTM (nn_Decoder).

Data-parallel over 8 NeuronCores: batch 4096 -> 512 per core. All weights
replicated. The T-1=127 step recurrence runs fully on-chip: enc_proj is
precomputed once into SBUF (bf16, [ENC, T, B] layout) and every step does
  hp   = 0.5*W1_h.T @ H + 0.5*W1_c.T @ C          (PE, H=2h, C=2c)
  X    = tanh(enc_proj + hp)                       (DVE add + ACT tanh->fp8)
  e    = w2.T @ X via fp8 DoubleRow matmuls        (PE, 2 timesteps/matmul)
  S    = exp(e/SW)                                 (ACT)
  den  = ones.T @ S ; num = ones.T @ (S*pfc)       (PE)
  r    = num / den                                 (DVE reciprocal + mult)
  gates= whh.T @ H + [wih;bias].T @ [yt;1]         (PE; per-gate scale folded)
  LSTM update via tanh-only form (single fused gate tanh on ACT)
Final output row: 0.5*Wfh.T @ H + (ones.T @ (S*pfin))/den + fc_final_b.
"""

import numpy as np
import ml_dtypes

import concourse.bass as bass
import concourse.bacc as bacc
import concourse.tile as tile
from concourse import mybir
from concourse.bass_utils import run_bass_kernel_spmd

NCORES = 8
B_FULL, T, E, D = 4096, 128, 128, 128
B = B_FULL // NCORES        # 512 batch per core
TSTEPS = T - 1              # 127
TC = 16                     # max t-chunk for the big add/tanh passes
# small chunks at sweep start (ACT starts sooner after hp) and end (less
# e-dot drain exposed in the serial tail)
CHUNKS = [4, 4, 8, 16, 16, 16, 16, 16, 16, 8, 4, 4]
assert sum(CHUNKS) == T
NBLK = B // 128             # 4 b-blocks of 128 for input transpose
SW = 16.0                   # fp8 scale on w2 (undone in the exp; tuned in _prep_host)
FP8_EDOT = True             # e-dot via fp8 DoubleRow (else bf16 one-hot)
MICROBENCH = False

FP = mybir.dt.float32
BF = mybir.dt.bfloat16
F8 = mybir.dt.float8e4
AF = mybir.ActivationFunctionType
OP = mybir.AluOpType
PM = mybir.MatmulPerfMode
BF_NP = ml_dtypes.bfloat16
F8_NP = ml_dtypes.float8_e4m3


def _build(fc_wy: float, fc_final_b: float, n_steps: int, sw: float = SW):
    nc = bacc.Bacc("TRN2", target_bir_lowering=False, debug=False,
                   num_devices=NCORES)

    x_ext = nc.declare_dram_parameter("x", [B, T, E], FP, isOutput=False)
    yh_ext = nc.declare_dram_parameter("yh", [TSTEPS, B], BF, isOutput=False)
    # [0.5*W1_h.T | 0.5*W1_c.T]  -> [D, 2E]
    w1hc_ext = nc.declare_dram_parameter("w1hc", [D, 2 * E], BF, isOutput=False)
    wke_ext = nc.declare_dram_parameter("wke", [E, E], BF, isOutput=False)  # W1_e.T
    # fp8 DoubleRow one-hot stationary: [:,0,T-1]=w2*SW, [:,1,T]=w2*SW
    w2g2_ext = nc.declare_dram_parameter("w2g2", [E, 2, 2 * T], F8,
                                         isOutput=False)
    w2g_ext = nc.declare_dram_parameter("w2g", [E, 2 * T], BF, isOutput=False)
    gfc_ext = nc.declare_dram_parameter("gfc", [E, 2 * T], BF, isOutput=False)
    gfin_ext = nc.declare_dram_parameter("gfin", [E, 2 * T], BF, isOutput=False)
    # per-gate scale folded: s_g*0.5*W_hh.T
    whh_ext = nc.declare_dram_parameter("whh", [D, 4 * D], BF, isOutput=False)
    # rank-2 gate tail: row0 = fc_wy*s_g*W_ih col, row1 = s_g*gate bias
    wb2_ext = nc.declare_dram_parameter("wb2", [2, 4 * D], BF, isOutput=False)
    wih1_ext = nc.declare_dram_parameter("wih1", [1, 4 * D], BF, isOutput=False)
    b1_ext = nc.declare_dram_parameter("b1", [E, 1], FP, isOutput=False)
    wfh_ext = nc.declare_dram_parameter("wfh", [D, 1], BF, isOutput=False)  # 0.5*Wfh
    id_ext = nc.declare_dram_parameter("ident", [128, 128], BF, isOutput=False)
    out_ext = nc.declare_dram_parameter("out", [1, B], FP, isOutput=True)

    with tile.TileContext(nc) as tc:
        import contextlib
        _stack = contextlib.ExitStack()
        const = _stack.enter_context(tc.tile_pool(name="const", bufs=1))
        dma4 = _stack.enter_context(tc.tile_pool(name="dma4", bufs=2))

        # ---- constants -------------------------------------------------
        w1hc_sb = const.tile([D, 2 * E], BF, tag="w1hc")
        nc.sync.dma_start(out=w1hc_sb[:], in_=w1hc_ext[:])

        if FP8_EDOT:
            w2g2_sb = const.tile([E, 2, 2 * T], F8, tag="w2g2")
            nc.sync.dma_start(out=w2g2_sb[:], in_=w2g2_ext[:])
        else:
            w2g_sb = const.tile([E, 2 * T], BF, tag="w2g")
            nc.sync.dma_start(out=w2g_sb[:], in_=w2g_ext[:])

        whh_sb = const.tile([D, 4 * D], BF, tag="whh")
        nc.sync.dma_start(out=whh_sb[:], in_=whh_ext[:])
        wb2_sb = const.tile([2, 4 * D], BF, tag="wb2")
        nc.sync.dma_start(out=wb2_sb[:], in_=wb2_ext[:])
        wih1_sb = const.tile([1, 4 * D], BF, tag="wih1")
        nc.sync.dma_start(out=wih1_sb[:], in_=wih1_ext[:])
        b1_sb = const.tile([E, 1], FP, tag="b1")
        nc.sync.dma_start(out=b1_sb[:], in_=b1_ext[:])
        wfh_sb = const.tile([D, 1], BF, tag="wfh")
        nc.sync.dma_start(out=wfh_sb[:], in_=wfh_ext[:])

        ones_sb = const.tile([T, 1], BF, tag="ones")
        nc.vector.memset(ones_sb[:], 1.0)
        ytones = const.tile([2, B], BF, tag="ytones")
        nc.vector.memset(ytones[:], 1.0)   # row0 overwritten per step

        encq0 = const.tile([E, 32, B], BF, tag="encq0")
        encq1 = const.tile([E, 32, B], BF, tag="encq1")
        encq2 = const.tile([E, 32, B], BF, tag="encq2")
        encq3 = const.tile([E, 32, B], BF, tag="encq3")
        encq = [encq0, encq1, encq2, encq3]
        pfc_sb = const.tile([T, B], BF, tag="pfc")
        pfin_sb = const.tile([T, B], BF, tag="pfin")
        C = const.tile([D, B], FP, tag="C")   # 2*c
        nc.vector.memset(C[:], 0.0)

        # ---- precompute: enc_proj, pfc, pfin ---------------------------
        with contextlib.ExitStack() as pre:
            pwork = pre.enter_context(tc.tile_pool(name="pwork", bufs=3))
            pdma = pre.enter_context(tc.tile_pool(name="pdma", bufs=6))
            pcst = pre.enter_context(tc.tile_pool(name="pcst", bufs=1))
            wke_sb = pcst.tile([E, E], BF, tag="wke")
            nc.sync.dma_start(out=wke_sb[:], in_=wke_ext[:])
            gfc_sb = pcst.tile([E, 2 * T], BF, tag="gfc")
            nc.sync.dma_start(out=gfc_sb[:], in_=gfc_ext[:])
            gfin_sb = pcst.tile([E, 2 * T], BF, tag="gfin")
            nc.sync.dma_start(out=gfin_sb[:], in_=gfin_ext[:])
            id_sb = pcst.tile([128, 128], BF, tag="ident")
            nc.sync.dma_start(out=id_sb[:], in_=id_ext[:])
            pps = pre.enter_context(tc.tile_pool(name="pps", bufs=4,
                                                 space="PSUM"))
            pps2 = pre.enter_context(tc.tile_pool(name="pps2", bufs=2,
                                                  space="PSUM"))
            pfc_ps = pps2.tile([T, B], FP, tag="p2")
            pfin_ps = pps2.tile([T, B], FP, tag="p2")
            for t in range(T):
                inT_ps = pps.tile([E, B], BF, tag="big")
                for blk in range(NBLK):
                    xin = pdma.tile([128, E], FP, tag="xin")
                    nc.sync.dma_start(
                        out=xin[:],
                        in_=x_ext[blk * 128:(blk + 1) * 128, t, :],
                    )
                    xbf = pwork.tile([128, E], BF, tag="xbf")
                    nc.vector.tensor_copy(xbf[:], xin[:])
                    nc.tensor.transpose(
                        inT_ps[:, blk * 128:(blk + 1) * 128], xbf[:], id_sb[:]
                    )
                inT = pwork.tile([E, B], BF, tag="inT")
                nc.vector.tensor_copy(inT[:], inT_ps[:])
                ep_ps = pps.tile([E, B], FP, tag="big")
                nc.tensor.matmul(ep_ps[:], wke_sb[:], inT[:],
                                 start=True, stop=True)
                nc.tensor.matmul(pfc_ps[:], gfc_sb[:, T - 1 - t:2 * T - 1 - t],
                                 inT[:], start=(t == 0), stop=(t == T - 1))
                nc.tensor.matmul(pfin_ps[:], gfin_sb[:, T - 1 - t:2 * T - 1 - t],
                                 inT[:], start=(t == 0), stop=(t == T - 1))
                # enc_proj + attn_b1, cast to bf16, store [E, t, B]
                nc.scalar.activation(encq[t // 32][:, t % 32, :], ep_ps[:],
                                     AF.Identity, bias=b1_sb[:], scale=1.0)
            nc.vector.tensor_copy(pfc_sb[:], pfc_ps[:])
            nc.vector.tensor_copy(pfin_sb[:], pfin_ps[:])

        # ---- main pools -----------------------------------------------
        xb = _stack.enter_context(tc.tile_pool(name="xb", bufs=2))
        xf = _stack.enter_context(tc.tile_pool(name="xf", bufs=2))
        wk = _stack.enter_context(tc.tile_pool(name="wk", bufs=2))
        wk1 = _stack.enter_context(tc.tile_pool(name="wk1", bufs=1))
        ps_e = _stack.enter_context(tc.tile_pool(name="ps_e", bufs=1,
                                                 space="PSUM"))
        ps_hp = _stack.enter_context(tc.tile_pool(name="ps_hp", bufs=1,
                                                  space="PSUM"))
        ps_g = _stack.enter_context(tc.tile_pool(name="ps_g", bufs=1,
                                                 space="PSUM"))
        ps_dn = _stack.enter_context(tc.tile_pool(name="ps_dn", bufs=2,
                                                  space="PSUM"))

        # initial bf16 state casts (zeros)
        Hbf = wk.tile([D, B], BF, tag="Hbf")
        Cbf = wk.tile([D, B], BF, tag="Cbf")
        nc.vector.memset(Hbf[:], 0.0)
        nc.vector.memset(Cbf[:], 0.0)

        rcp = None
        S_sb = None

        # ---- the recurrence -------------------------------------------
        for s in range(n_steps):
            nc.sync.dma_start(out=ytones[0:1, :], in_=yh_ext[s:s + 1, :])
            if s > 0:
                # hp = 0.5*W1h.T @ H + 0.5*W1c.T @ C   [E, B]
                hp_ps = ps_hp.tile([E, B], FP, tag="hp")
                nc.tensor.matmul(hp_ps[:], w1hc_sb[:, E:2 * E], Cbf[:],
                                 start=True, stop=False)
                nc.tensor.matmul(hp_ps[:], w1hc_sb[:, 0:E], Hbf[:],
                                 start=False, stop=True)
                hp_sb = wk1.tile([E, B], BF, tag="hp_sb")
                nc.vector.tensor_copy(hp_sb[:], hp_ps[:])
            # gates parts 1+2 (hoisted out of the serial tail):
            #   s_g*0.5*Whh.T @ H  +  [fc_wy*s_g*wih; s_g*bias].T @ [y; 1]
            g_ps = ps_g.tile([D, 4, B], FP, tag="g")
            for g in range(4):
                nc.tensor.matmul(g_ps[:, g, :], whh_sb[:, g * D:(g + 1) * D],
                                 Hbf[:], start=True, stop=False)
                nc.tensor.matmul(g_ps[:, g, :], wb2_sb[:, g * D:(g + 1) * D],
                                 ytones[:], start=False, stop=False)

            e_ps = ps_e.tile([T, B], FP, tag="e")
            t0c = 0
            for csz in CHUNKS:
                eq = encq[t0c // 32]
                eqs = eq[:, t0c % 32:t0c % 32 + csz, :]
                if s > 0:
                    Xb = xb.tile([E, TC, B], BF, tag="Xb")
                    hp_c = hp_sb[:].unsqueeze(1).broadcast_to([E, csz, B])
                    nc.vector.tensor_tensor(Xb[:, 0:csz, :], eqs, hp_c,
                                            op=OP.add)
                    tanh_in = Xb[:, 0:csz, :]
                else:
                    # s == 0: h = c = 0, so hp == 0 and X = tanh(enc_proj)
                    tanh_in = eqs
                if FP8_EDOT:
                    Xf = xf.tile([E, TC, B], F8, tag="Xf")
                    nc.scalar.activation(Xf[:, 0:csz, :], tanh_in,
                                         AF.Tanh)
                    for j in range(0, csz, 2):
                        t = t0c + j
                        nc.tensor.matmul(
                            e_ps[:], w2g2_sb[:, :, T - 1 - t:2 * T - 1 - t],
                            Xf[:, j:j + 2, :],
                            start=(t == 0), stop=(t == T - 2),
                            perf_mode=PM.DoubleRow)
                else:
                    Xg = xb.tile([E, TC, B], BF, tag="Xb")
                    nc.scalar.activation(Xg[:, 0:csz, :], tanh_in, AF.Tanh)
                    for j in range(csz):
                        t = t0c + j
                        nc.tensor.matmul(
                            e_ps[:], w2g_sb[:, T - 1 - t:2 * T - 1 - t],
                            Xg[:, j, :], start=(t == 0), stop=(t == T - 1))
                t0c += csz

            S_sb = wk1.tile([T, B], BF, tag="S")
            nc.scalar.activation(S_sb[:], e_ps[:], AF.Exp,
                                 scale=(1.0 / sw) if FP8_EDOT else 1.0)
            den_ps = ps_dn.tile([1, B], FP, tag="p2")
            nc.tensor.matmul(den_ps[:], ones_sb[:], S_sb[:],
                             start=True, stop=True)
            SP = wk1.tile([T, B], BF, tag="SP")
            nc.vector.tensor_tensor(SP[:], S_sb[:], pfc_sb[:], op=OP.mult)
            num_ps = ps_dn.tile([1, B], FP, tag="p2")
            nc.tensor.matmul(num_ps[:], ones_sb[:], SP[:],
                             start=True, stop=True)

            rr = wk1.tile([1, 2 * B], FP, tag="rr")
            rcp = rr[:, 0:B]
            nc.vector.reciprocal_approx_fast(rcp, den_ps[:])
            rb = wk1.tile([1, B], BF, tag="rb")
            nc.vector.tensor_tensor(rb[:], num_ps[:], rcp, op=OP.mult)

            # gates part 3 (in-chain): + s_g*wih (x) r
            for g in range(4):
                nc.tensor.matmul(g_ps[:, g, :], wih1_sb[:, g * D:(g + 1) * D],
                                 rb[:], start=False, stop=True)
            tg = wk1.tile([D, 4, B], BF, tag="tg")
            nc.scalar.activation(tg[:], g_ps[:], AF.Tanh)

            # C_new(=2c) = 0.5*(tf+1)*C + (ti+1)*tg ; H_new(=2h) = (to+1)*tanh(c)
            tmp1 = wk1.tile([D, B], FP, tag="tmp1")
            nc.vector.scalar_tensor_tensor(tmp1[:], tg[:, 1, :], 1.0, C[:],
                                           op0=OP.add, op1=OP.mult)
            tmp2 = wk1.tile([D, B], BF, tag="tmp2")
            nc.vector.scalar_tensor_tensor(tmp2[:], tg[:, 0, :], 1.0,
                                           tg[:, 2, :],
                                           op0=OP.add, op1=OP.mult)
            nc.vector.scalar_tensor_tensor(C[:], tmp1[:], 0.5, tmp2[:],
                                           op0=OP.mult, op1=OP.add)
            Cbf = wk.tile([D, B], BF, tag="Cbf")
            nc.vector.tensor_copy(Cbf[:], C[:])
            tct = wk1.tile([D, B], BF, tag="tct")
            nc.scalar.activation(tct[:], C[:], AF.Tanh, scale=0.5)
            Hbf = wk.tile([D, B], BF, tag="Hbf")
            nc.vector.scalar_tensor_tensor(Hbf[:], tg[:, 3, :], 1.0, tct[:],
                                           op0=OP.add, op1=OP.mult)

        # ---- final output row ----------------------------------------
        o_ps = ps_dn.tile([1, B], FP, tag="p2")
        nc.tensor.matmul(o_ps[:], wfh_sb[:], Hbf[:], start=True, stop=True)
        if n_steps > 0:
            SPf = wk1.tile([T, B], BF, tag="SP")
            nc.vector.tensor_tensor(SPf[:], S_sb[:], pfin_sb[:], op=OP.mult)
            nf_ps = ps_dn.tile([1, B], FP, tag="p2")
            nc.tensor.matmul(nf_ps[:], ones_sb[:], SPf[:], start=True, stop=True)
            rfin = xb.tile([1, B], FP, tag="Xb")
            nc.vector.tensor_tensor(rfin[:], nf_ps[:], rcp, op=OP.mult)
            o_sb = xb.tile([1, B], FP, tag="Xb")
            nc.vector.scalar_tensor_tensor(o_sb[:], o_ps[:], fc_final_b,
                                           rfin[:], op0=OP.add, op1=OP.add)
        else:
            o_sb = xb.tile([1, B], FP, tag="Xb")
            nc.vector.tensor_scalar_add(o_sb[:], o_ps[:], fc_final_b)
        nc.sync.dma_start(out=out_ext[:], in_=o_sb[:])

        _stack.close()

    nc.finalize()
    return nc


def _prep_host(inputs, n_steps):
    f32 = np.float32
    attn_W1 = np.asarray(inputs["attn_W1"], f32)
    attn_W2 = np.asarray(inputs["attn_W2"], f32)
    W_ih = np.asarray(inputs["W_ih"], f32)
    W_hh = np.asarray(inputs["W_hh"], f32)
    b_ih = np.asarray(inputs["b_ih"], f32)
    b_hh = np.asarray(inputs["b_hh"], f32)
    fc_W = np.asarray(inputs["fc_W"], f32)
    fc_b = np.asarray(inputs["fc_b"], f32)
    fcf_W = np.asarray(inputs["fc_final_W"], f32)
    fcf_b = np.asarray(inputs["fc_final_b"], f32)

    W1_h = attn_W1[:, :D]
    W1_c = attn_W1[:, D:2 * D]
    W1_e = attn_W1[:, 2 * D:]

    w1hc = np.concatenate([0.5 * W1_h.T, 0.5 * W1_c.T], axis=1)      # [D, 2E]
    wke = np.ascontiguousarray(W1_e.T)                                # [E, E]
    def onehot_shift(vec):
        g = np.zeros((E, 2 * T), f32)
        g[:, T - 1] = vec
        return g.astype(BF_NP)
    w2g = onehot_shift(attn_W2[0])
    # pick the fp8 scale that minimizes w2 quantization error
    best_sw, best_err = SW, np.inf
    for sw_c in (8.0, 12.0, 16.0, 24.0, 32.0, 48.0, 64.0, 96.0):
        deq = np.asarray(attn_W2[0] * sw_c, F8_NP).astype(f32) / sw_c
        err = float(np.sum((deq - attn_W2[0]) ** 2))
        if err < best_err:
            best_sw, best_err = sw_c, err
    sw = best_sw
    w2g2 = np.zeros((E, 2, 2 * T), f32)
    w2g2[:, 0, T - 1] = attn_W2[0] * sw
    w2g2[:, 1, T] = attn_W2[0] * sw
    gfc = onehot_shift(fc_W[0, :E])
    gfin = onehot_shift(fcf_W[0, D:])
    # per-gate tanh input scale (tanh-only LSTM form), folded into weights
    scales = np.array([0.5, 0.5, 1.0, 0.5], f32)
    sg = np.repeat(scales, D)                                         # [4D]
    whh = 0.5 * W_hh.T * sg[None, :]                                  # [D, 4D]
    wih_row = W_ih[:, 0] * sg                                         # [4D]
    bias_row = (b_ih + b_hh + W_ih[:, 0] * float(fc_b[0])) * sg       # [4D]
    fc_wy = float(fc_W[0, E])
    wb2 = np.stack([wih_row * fc_wy, bias_row], axis=0)               # [2, 4D]
    wfh = 0.5 * fcf_W[0, :D][:, None]                                 # [D, 1]
    b1 = np.asarray(inputs["attn_b1"], f32)[:, None]

    weights = {
        "w1hc": w1hc.astype(BF_NP), "wke": wke.astype(BF_NP),
        "w2g": w2g, "w2g2": w2g2.astype(F8_NP),
        "gfc": gfc, "gfin": gfin, "whh": whh.astype(BF_NP),
        "wb2": wb2.astype(BF_NP), "wih1": wih_row[None, :].astype(BF_NP),
        "b1": b1.astype(f32),
        "wfh": wfh.astype(BF_NP),
        "ident": np.eye(128, dtype=f32).astype(BF_NP),
    }

    x_full = np.ascontiguousarray(np.asarray(inputs["input_encoded"], f32))
    yh_full = np.asarray(inputs["y_history"], f32)[:, :, 0]           # [B_FULL, 127]

    in_maps = []
    for i in range(NCORES):
        sl = slice(i * B, (i + 1) * B)
        m = dict(weights)
        m["x"] = x_full[sl]
        m["yh"] = np.ascontiguousarray(yh_full[sl].T).astype(BF_NP)   # [127, B]
        in_maps.append(m)
    return in_maps, fc_wy, float(fcf_b[0]), sw


_RUN_KW = {}


def _kernel_impl(inputs, n_steps):
    in_maps, fc_wy, fcf_b, sw = _prep_host(inputs, n_steps)
    nc = _build(fc_wy, fcf_b, n_steps, sw)
    res = run_bass_kernel_spmd(nc, in_maps, core_ids=list(range(NCORES)),
                               **_RUN_KW)
    out = np.concatenate(
        [np.asarray(res.results[i]["out"], np.float32).reshape(B, 1)
         for i in range(NCORES)], axis=0)
    return out, res


def kernel(**inputs) -> np.ndarray:
    out, _ = _kernel_impl(inputs, TSTEPS)
    return out


# revision 30
# speedup vs baseline: 1.1859x; 1.1859x over previous
"""Trainium2 Bass kernel for an attention-decoder LSTM (nn_Decoder).

Data-parallel over 8 NeuronCores: batch 4096 -> 512 per core. All weights
replicated. The T-1=127 step recurrence runs fully on-chip: enc_proj is
precomputed once into SBUF (bf16, [ENC, T, B] layout) and every step does
  hp   = 0.5*W1_h.T @ H + 0.5*W1_c.T @ C          (PE, H=2h, C=2c)
  X    = tanh(enc_proj + hp)                       (DVE add + ACT tanh->fp8)
  e    = w2.T @ X via fp8 DoubleRow matmuls        (PE, 2 timesteps/matmul)
  S    = exp(e/SW)                                 (ACT)
  den  = ones.T @ S ; num = ones.T @ (S*pfc)       (PE)
  r    = num / den                                 (DVE reciprocal + mult)
  gates= whh.T @ H + [wih;bias].T @ [yt;1]         (PE; per-gate scale folded)
  LSTM update via tanh-only form (single fused gate tanh on ACT)
Final output row: 0.5*Wfh.T @ H + (ones.T @ (S*pfin))/den + fc_final_b.
"""

import numpy as np
import ml_dtypes

import concourse.bass as bass
import concourse.bacc as bacc
import concourse.tile as tile
from concourse import mybir
from concourse.bass_utils import run_bass_kernel_spmd

NCORES = 8
B_FULL, T, E, D = 4096, 128, 128, 128
B = B_FULL // NCORES        # 512 batch per core
TSTEPS = T - 1              # 127
TC = 16                     # max t-chunk for the big add/tanh passes
# small chunks at sweep start (ACT starts sooner after hp) and end (less
# e-dot drain exposed in the serial tail)
CHUNKS = [4, 4, 8, 16, 16, 16, 16, 16, 16, 8, 4, 4]
assert sum(CHUNKS) == T
NBLK = B // 128             # 4 b-blocks of 128 for input transpose
SW = 16.0                   # fp8 scale on w2 (undone in the exp; tuned in _prep_host)
FP8_EDOT = True             # e-dot via fp8 DoubleRow (else bf16 one-hot)
MICROBENCH = False

FP = mybir.dt.float32
BF = mybir.dt.bfloat16
F8 = mybir.dt.float8e4
AF = mybir.ActivationFunctionType
OP = mybir.AluOpType
PM = mybir.MatmulPerfMode
BF_NP = ml_dtypes.bfloat16
F8_NP = ml_dtypes.float8_e4m3


def _build(fc_wy: float, fc_final_b: float, n_steps: int, sw: float = SW):
    nc = bacc.Bacc("TRN2", target_bir_lowering=False, debug=False,
                   num_devices=NCORES)

    x_ext = nc.declare_dram_parameter("x", [B, T, E], FP, isOutput=False)
    yh_ext = nc.declare_dram_parameter("yh", [TSTEPS, B], BF, isOutput=False)
    # [0.5*W1_h.T | 0.5*W1_c.T]  -> [D, 2E]
    w1hc_ext = nc.declare_dram_parameter("w1hc", [D, 2 * E], BF, isOutput=False)
    wke_ext = nc.declare_dram_parameter("wke", [E, E], BF, isOutput=False)  # W1_e.T
    # fp8 DoubleRow one-hot stationary: [:,0,T-1]=w2*SW, [:,1,T]=w2*SW
    w2g2_ext = nc.declare_dram_parameter("w2g2", [E, 2, 2 * T], F8,
                                         isOutput=False)
    w2g_ext = nc.declare_dram_parameter("w2g", [E, 2 * T], BF, isOutput=False)
    gfc_ext = nc.declare_dram_parameter("gfc", [E, 2 * T], BF, isOutput=False)
    gfin_ext = nc.declare_dram_parameter("gfin", [E, 2 * T], BF, isOutput=False)
    # per-gate scale folded: s_g*0.5*W_hh.T
    whh_ext = nc.declare_dram_parameter("whh", [D, 4 * D], BF, isOutput=False)
    # rank-2 gate tail: row0 = fc_wy*s_g*W_ih col, row1 = s_g*gate bias
    wb2_ext = nc.declare_dram_parameter("wb2", [2, 4 * D], BF, isOutput=False)
    wih1_ext = nc.declare_dram_parameter("wih1", [1, 4 * D], BF, isOutput=False)
    b1_ext = nc.declare_dram_parameter("b1", [E, 1], FP, isOutput=False)
    wfh_ext = nc.declare_dram_parameter("wfh", [D, 1], BF, isOutput=False)  # 0.5*Wfh
    id_ext = nc.declare_dram_parameter("ident", [128, 128], BF, isOutput=False)
    out_ext = nc.declare_dram_parameter("out", [1, B], FP, isOutput=True)

    with tile.TileContext(nc) as tc:
        import contextlib
        _stack = contextlib.ExitStack()
        const = _stack.enter_context(tc.tile_pool(name="const", bufs=1))
        dma4 = _stack.enter_context(tc.tile_pool(name="dma4", bufs=2))

        # ---- constants -------------------------------------------------
        w1hc_sb = const.tile([D, 2 * E], BF, tag="w1hc")
        nc.sync.dma_start(out=w1hc_sb[:], in_=w1hc_ext[:])

        if FP8_EDOT:
            w2g2_sb = const.tile([E, 2, 2 * T], F8, tag="w2g2")
            nc.sync.dma_start(out=w2g2_sb[:], in_=w2g2_ext[:])
        else:
            w2g_sb = const.tile([E, 2 * T], BF, tag="w2g")
            nc.sync.dma_start(out=w2g_sb[:], in_=w2g_ext[:])

        whh_sb = const.tile([D, 4 * D], BF, tag="whh")
        nc.sync.dma_start(out=whh_sb[:], in_=whh_ext[:])
        wb2_sb = const.tile([2, 4 * D], BF, tag="wb2")
        nc.sync.dma_start(out=wb2_sb[:], in_=wb2_ext[:])
        wih1_sb = const.tile([1, 4 * D], BF, tag="wih1")
        nc.sync.dma_start(out=wih1_sb[:], in_=wih1_ext[:])
        b1_sb = const.tile([E, 1], FP, tag="b1")
        nc.sync.dma_start(out=b1_sb[:], in_=b1_ext[:])
        wfh_sb = const.tile([D, 1], BF, tag="wfh")
        nc.sync.dma_start(out=wfh_sb[:], in_=wfh_ext[:])

        ones_sb = const.tile([T, 1], BF, tag="ones")
        nc.vector.memset(ones_sb[:], 1.0)
        ytones = const.tile([2, B], BF, tag="ytones")
        nc.vector.memset(ytones[:], 1.0)   # row0 overwritten per step

        encq0 = const.tile([E, 32, B], BF, tag="encq0")
        encq1 = const.tile([E, 32, B], BF, tag="encq1")
        encq2 = const.tile([E, 32, B], BF, tag="encq2")
        encq3 = const.tile([E, 32, B], BF, tag="encq3")
        encq = [encq0, encq1, encq2, encq3]
        pfc_sb = const.tile([T, B], BF, tag="pfc")
        pfin_sb = const.tile([T, B], BF, tag="pfin")
        C = const.tile([D, B], FP, tag="C")   # 2*c
        nc.vector.memset(C[:], 0.0)

        # ---- precompute: enc_proj, pfc, pfin ---------------------------
        with contextlib.ExitStack() as pre:
            pwork = pre.enter_context(tc.tile_pool(name="pwork", bufs=3))
            pdma = pre.enter_context(tc.tile_pool(name="pdma", bufs=6))
            pcst = pre.enter_context(tc.tile_pool(name="pcst", bufs=1))
            wke_sb = pcst.tile([E, E], BF, tag="wke")
            nc.sync.dma_start(out=wke_sb[:], in_=wke_ext[:])
            gfc_sb = pcst.tile([E, 2 * T], BF, tag="gfc")
            nc.sync.dma_start(out=gfc_sb[:], in_=gfc_ext[:])
            gfin_sb = pcst.tile([E, 2 * T], BF, tag="gfin")
            nc.sync.dma_start(out=gfin_sb[:], in_=gfin_ext[:])
            id_sb = pcst.tile([128, 128], BF, tag="ident")
            nc.sync.dma_start(out=id_sb[:], in_=id_ext[:])
            pps = pre.enter_context(tc.tile_pool(name="pps", bufs=4,
                                                 space="PSUM"))
            pps2 = pre.enter_context(tc.tile_pool(name="pps2", bufs=2,
                                                  space="PSUM"))
            pfc_ps = pps2.tile([T, B], FP, tag="p2")
            pfin_ps = pps2.tile([T, B], FP, tag="p2")
            for t in range(T):
                inT_ps = pps.tile([E, B], BF, tag="big")
                for blk in range(NBLK):
                    xin = pdma.tile([128, E], FP, tag="xin")
                    nc.sync.dma_start(
                        out=xin[:],
                        in_=x_ext[blk * 128:(blk + 1) * 128, t, :],
                    )
                    xbf = pwork.tile([128, E], BF, tag="xbf")
                    nc.vector.tensor_copy(xbf[:], xin[:])
                    nc.tensor.transpose(
                        inT_ps[:, blk * 128:(blk + 1) * 128], xbf[:], id_sb[:]
                    )
                inT = pwork.tile([E, B], BF, tag="inT")
                nc.vector.tensor_copy(inT[:], inT_ps[:])
                ep_ps = pps.tile([E, B], FP, tag="big")
                nc.tensor.matmul(ep_ps[:], wke_sb[:], inT[:],
                                 start=True, stop=True)
                nc.tensor.matmul(pfc_ps[:], gfc_sb[:, T - 1 - t:2 * T - 1 - t],
                                 inT[:], start=(t == 0), stop=(t == T - 1))
                nc.tensor.matmul(pfin_ps[:], gfin_sb[:, T - 1 - t:2 * T - 1 - t],
                                 inT[:], start=(t == 0), stop=(t == T - 1))
                # enc_proj + attn_b1, cast to bf16, store [E, t, B]
                nc.scalar.activation(encq[t // 32][:, t % 32, :], ep_ps[:],
                                     AF.Identity, bias=b1_sb[:], scale=1.0)
            nc.vector.tensor_copy(pfc_sb[:], pfc_ps[:])
            nc.vector.tensor_copy(pfin_sb[:], pfin_ps[:])

        # ---- main pools -----------------------------------------------
        xb = _stack.enter_context(tc.tile_pool(name="xb", bufs=2))
        xf = _stack.enter_context(tc.tile_pool(name="xf", bufs=2))
        wk = _stack.enter_context(tc.tile_pool(name="wk", bufs=2))
        wk1 = _stack.enter_context(tc.tile_pool(name="wk1", bufs=1))
        ps_e = _stack.enter_context(tc.tile_pool(name="ps_e", bufs=1,
                                                 space="PSUM"))
        ps_hp = _stack.enter_context(tc.tile_pool(name="ps_hp", bufs=1,
                                                  space="PSUM"))
        ps_g = _stack.enter_context(tc.tile_pool(name="ps_g", bufs=1,
                                                 space="PSUM"))
        ps_dn = _stack.enter_context(tc.tile_pool(name="ps_dn", bufs=2,
                                                  space="PSUM"))

        # initial bf16 state casts (zeros)
        Hbf = wk.tile([D, B], BF, tag="Hbf")
        Cbf = wk.tile([D, B], BF, tag="Cbf")
        nc.vector.memset(Hbf[:], 0.0)
        nc.vector.memset(Cbf[:], 0.0)

        rcp = None
        S_sb = None

        # ---- the recurrence -------------------------------------------
        for s in range(n_steps):
            nc.sync.dma_start(out=ytones[0:1, :], in_=yh_ext[s:s + 1, :])
            if s > 0:
                # hp = 0.5*W1h.T @ H + 0.5*W1c.T @ C   [E, B]
                hp_ps = ps_hp.tile([E, B], FP, tag="hp")
                nc.tensor.matmul(hp_ps[:], w1hc_sb[:, 0:E], Hbf[:],
                                 start=True, stop=False)
                nc.tensor.matmul(hp_ps[:], w1hc_sb[:, E:2 * E], Cbf[:],
                                 start=False, stop=True)
                hp_sb = wk1.tile([E, B], BF, tag="hp_sb")
                nc.vector.tensor_copy(hp_sb[:], hp_ps[:])
            # gates parts 1+2 (hoisted out of the serial tail):
            #   s_g*0.5*Whh.T @ H  +  [fc_wy*s_g*wih; s_g*bias].T @ [y; 1]
            g_ps = ps_g.tile([D, 4, B], FP, tag="g")
            for g in range(4):
                nc.tensor.matmul(g_ps[:, g, :], whh_sb[:, g * D:(g + 1) * D],
                                 Hbf[:], start=True, stop=False)
                nc.tensor.matmul(g_ps[:, g, :], wb2_sb[:, g * D:(g + 1) * D],
                                 ytones[:], start=False, stop=False)

            e_ps = ps_e.tile([T, B], FP, tag="e")
            t0c = 0
            for csz in CHUNKS:
                eq = encq[t0c // 32]
                eqs = eq[:, t0c % 32:t0c % 32 + csz, :]
                if s > 0:
                    Xb = xb.tile([E, TC, B], BF, tag="Xb")
                    hp_c = hp_sb[:].unsqueeze(1).broadcast_to([E, csz, B])
                    nc.vector.tensor_tensor(Xb[:, 0:csz, :], eqs, hp_c,
                                            op=OP.add)
                    tanh_in = Xb[:, 0:csz, :]
                else:
                    # s == 0: h = c = 0, so hp == 0 and X = tanh(enc_proj)
                    tanh_in = eqs
                if FP8_EDOT:
                    Xf = xf.tile([E, TC, B], F8, tag="Xf")
                    nc.scalar.activation(Xf[:, 0:csz, :], tanh_in,
                                         AF.Tanh)
                    for j in range(0, csz, 2):
                        t = t0c + j
                        nc.tensor.matmul(
                            e_ps[:], w2g2_sb[:, :, T - 1 - t:2 * T - 1 - t],
                            Xf[:, j:j + 2, :],
                            start=(t == 0), stop=(t == T - 2),
                            perf_mode=PM.DoubleRow)
                else:
                    Xg = xb.tile([E, TC, B], BF, tag="Xb")
                    nc.scalar.activation(Xg[:, 0:csz, :], tanh_in, AF.Tanh)
                    for j in range(csz):
                        t = t0c + j
                        nc.tensor.matmul(
                            e_ps[:], w2g_sb[:, T - 1 - t:2 * T - 1 - t],
                            Xg[:, j, :], start=(t == 0), stop=(t == T - 1))
                t0c += csz

            S_sb = wk1.tile([T, B], BF, tag="S")
            nc.scalar.activation(S_sb[:], e_ps[:], AF.Exp,
                                 scale=(1.0 / sw) if FP8_EDOT else 1.0)
            den_ps = ps_dn.tile([1, B], FP, tag="p2")
            nc.tensor.matmul(den_ps[:], ones_sb[:], S_sb[:],
                             start=True, stop=True)
            SP = wk1.tile([T, B], BF, tag="SP")
            nc.vector.tensor_tensor(SP[:], S_sb[:], pfc_sb[:], op=OP.mult)
            num_ps = ps_dn.tile([1, B], FP, tag="p2")
            nc.tensor.matmul(num_ps[:], ones_sb[:], SP[:],
                             start=True, stop=True)

            rr = wk1.tile([1, 2 * B], FP, tag="rr")
            rcp = rr[:, 0:B]
            nc.vector.reciprocal_approx_fast(rcp, den_ps[:])
            rb = wk1.tile([1, B], BF, tag="rb")
            nc.vector.tensor_tensor(rb[:], num_ps[:], rcp, op=OP.mult)

            # gates part 3 (in-chain): + s_g*wih (x) r
            for g in range(4):
                nc.tensor.matmul(g_ps[:, g, :], wih1_sb[:, g * D:(g + 1) * D],
                                 rb[:], start=False, stop=True)
            tg = wk1.tile([D, 4, B], BF, tag="tg")
            nc.scalar.activation(tg[:], g_ps[:], AF.Tanh)

            # C_new(=2c) = 0.5*(tf+1)*C + (ti+1)*tg ; H_new(=2h) = (to+1)*tanh(c)
            tmp1 = wk1.tile([D, B], FP, tag="tmp1")
            nc.vector.scalar_tensor_tensor(tmp1[:], tg[:, 1, :], 1.0, C[:],
                                           op0=OP.add, op1=OP.mult)
            tmp2 = wk1.tile([D, B], BF, tag="tmp2")
            nc.vector.scalar_tensor_tensor(tmp2[:], tg[:, 0, :], 1.0,
                                           tg[:, 2, :],
                                           op0=OP.add, op1=OP.mult)
            nc.vector.scalar_tensor_tensor(C[:], tmp1[:], 0.5, tmp2[:],
                                           op0=OP.mult, op1=OP.add)
            tct = wk1.tile([D, B], BF, tag="tct")
            nc.scalar.activation(tct[:], C[:], AF.Tanh, scale=0.5)
            Hbf = wk.tile([D, B], BF, tag="Hbf")
            nc.vector.scalar_tensor_tensor(Hbf[:], tg[:, 3, :], 1.0, tct[:],
                                           op0=OP.add, op1=OP.mult)
            Cbf = wk.tile([D, B], BF, tag="Cbf")
            nc.vector.tensor_copy(Cbf[:], C[:])

        # ---- final output row ----------------------------------------
        o_ps = ps_dn.tile([1, B], FP, tag="p2")
        nc.tensor.matmul(o_ps[:], wfh_sb[:], Hbf[:], start=True, stop=True)
        if n_steps > 0:
            SPf = wk1.tile([T, B], BF, tag="SP")
            nc.vector.tensor_tensor(SPf[:], S_sb[:], pfin_sb[:], op=OP.mult)
            nf_ps = ps_dn.tile([1, B], FP, tag="p2")
            nc.tensor.matmul(nf_ps[:], ones_sb[:], SPf[:], start=True, stop=True)
            rfin = xb.tile([1, B], FP, tag="Xb")
            nc.vector.tensor_tensor(rfin[:], nf_ps[:], rcp, op=OP.mult)
            o_sb = xb.tile([1, B], FP, tag="Xb")
            nc.vector.scalar_tensor_tensor(o_sb[:], o_ps[:], fc_final_b,
                                           rfin[:], op0=OP.add, op1=OP.add)
        else:
            o_sb = xb.tile([1, B], FP, tag="Xb")
            nc.vector.tensor_scalar_add(o_sb[:], o_ps[:], fc_final_b)
        nc.sync.dma_start(out=out_ext[:], in_=o_sb[:])

        _stack.close()

    nc.finalize()
    return nc


def _prep_host(inputs, n_steps):
    f32 = np.float32
    attn_W1 = np.asarray(inputs["attn_W1"], f32)
    attn_W2 = np.asarray(inputs["attn_W2"], f32)
    W_ih = np.asarray(inputs["W_ih"], f32)
    W_hh = np.asarray(inputs["W_hh"], f32)
    b_ih = np.asarray(inputs["b_ih"], f32)
    b_hh = np.asarray(inputs["b_hh"], f32)
    fc_W = np.asarray(inputs["fc_W"], f32)
    fc_b = np.asarray(inputs["fc_b"], f32)
    fcf_W = np.asarray(inputs["fc_final_W"], f32)
    fcf_b = np.asarray(inputs["fc_final_b"], f32)

    W1_h = attn_W1[:, :D]
    W1_c = attn_W1[:, D:2 * D]
    W1_e = attn_W1[:, 2 * D:]

    w1hc = np.concatenate([0.5 * W1_h.T, 0.5 * W1_c.T], axis=1)      # [D, 2E]
    wke = np.ascontiguousarray(W1_e.T)                                # [E, E]
    def onehot_shift(vec):
        g = np.zeros((E, 2 * T), f32)
        g[:, T - 1] = vec
        return g.astype(BF_NP)
    w2g = onehot_shift(attn_W2[0])
    # pick the fp8 scale that minimizes w2 quantization error
    best_sw, best_err = SW, np.inf
    for sw_c in (8.0, 12.0, 16.0, 24.0, 32.0, 48.0, 64.0, 96.0):
        deq = np.asarray(attn_W2[0] * sw_c, F8_NP).astype(f32) / sw_c
        err = float(np.sum((deq - attn_W2[0]) ** 2))
        if err < best_err:
            best_sw, best_err = sw_c, err
    sw = best_sw
    w2g2 = np.zeros((E, 2, 2 * T), f32)
    w2g2[:, 0, T - 1] = attn_W2[0] * sw
    w2g2[:, 1, T] = attn_W2[0] * sw
    gfc = onehot_shift(fc_W[0, :E])
    gfin = onehot_shift(fcf_W[0, D:])
    # per-gate tanh input scale (tanh-only LSTM form), folded into weights
    scales = np.array([0.5, 0.5, 1.0, 0.5], f32)
    sg = np.repeat(scales, D)                                         # [4D]
    whh = 0.5 * W_hh.T * sg[None, :]                                  # [D, 4D]
    wih_row = W_ih[:, 0] * sg                                         # [4D]
    bias_row = (b_ih + b_hh + W_ih[:, 0] * float(fc_b[0])) * sg       # [4D]
    fc_wy = float(fc_W[0, E])
    wb2 = np.stack([wih_row * fc_wy, bias_row], axis=0)               # [2, 4D]
    wfh = 0.5 * fcf_W[0, :D][:, None]                                 # [D, 1]
    b1 = np.asarray(inputs["attn_b1"], f32)[:, None]

    weights = {
        "w1hc": w1hc.astype(BF_NP), "wke": wke.astype(BF_NP),
        "w2g": w2g, "w2g2": w2g2.astype(F8_NP),
        "gfc": gfc, "gfin": gfin, "whh": whh.astype(BF_NP),
        "wb2": wb2.astype(BF_NP), "wih1": wih_row[None, :].astype(BF_NP),
        "b1": b1.astype(f32),
        "wfh": wfh.astype(BF_NP),
        "ident": np.eye(128, dtype=f32).astype(BF_NP),
    }

    x_full = np.ascontiguousarray(np.asarray(inputs["input_encoded"], f32))
    yh_full = np.asarray(inputs["y_history"], f32)[:, :, 0]           # [B_FULL, 127]

    in_maps = []
    for i in range(NCORES):
        sl = slice(i * B, (i + 1) * B)
        m = dict(weights)
        m["x"] = x_full[sl]
        m["yh"] = np.ascontiguousarray(yh_full[sl].T).astype(BF_NP)   # [127, B]
        in_maps.append(m)
    return in_maps, fc_wy, float(fcf_b[0]), sw


_RUN_KW = {}


def _kernel_impl(inputs, n_steps):
    in_maps, fc_wy, fcf_b, sw = _prep_host(inputs, n_steps)
    nc = _build(fc_wy, fcf_b, n_steps, sw)
    res = run_bass_kernel_spmd(nc, in_maps, core_ids=list(range(NCORES)),
                               **_RUN_KW)
    out = np.concatenate(
        [np.asarray(res.results[i]["out"], np.float32).reshape(B, 1)
         for i in range(NCORES)], axis=0)
    return out, res


def kernel(**inputs) -> np.ndarray:
    out, _ = _kernel_impl(inputs, TSTEPS)
    return out
